# revision 16
# baseline (speedup 1.0000x reference)
"""Trainium2 Bass kernel for MultiHeadSelfAttention (K-only variant).

Math (per batch b):
    K  = x @ Wk.T;  Kh = heads(K)
    S_h = Kh @ Kh.T / sqrt(D);  P_h = softmax(S_h)
    wV_h = P_h @ Kh  (V == K);  out = concat_h(wV) @ Wo.T

Sharding (8 cores): core c handles batch c//2 and query-half c%2 with all
heads.  The query half is selected by rolling x on the host so each core
always computes queries 0:S//2 of its (rolled) sequence; softmax over keys
is order-invariant so rolling the key axis is harmless.

Per-core pipeline (one SPMD NEFF):
    xT_bf  = XBAR-DMA-transpose(bf16(x))     [d, s]
    K      = xT_bf.T @ WkT_bf  (bf16 matmuls, fp32 psum) -> k_bf, kones
    khT    = XBAR-DMA-transpose(k_bf)        [e, s] bf16
    per (qb, head):
      scores strip S_T[k, q] = khT_h.T @ khT_h[:, qb]    (bf16)
      E_T = exp(S_T / sqrt(D))       (ScalarE, psum -> sbuf bf16)
      PV:  [wVT_h ; rowsum_h] = [Kh_h | 1].T @ E_T       (bf16, psum accum)
      recip = 1/rowsum; partition-broadcast via K=1 matmul; normalize wVT
    out = wVTn.T @ WoT  (fp32r), first half overlapped with second qb
"""

import sys

if "/opt/trn_rl_repo" not in sys.path:
    sys.path.insert(0, "/opt/trn_rl_repo")

import numpy as np

B, S, D = 4, 2048, 512
H = 8
HD = D // H            # 64
P = 128
SH = S // 2            # query half per core = 1024
NCORES = 8
SCALE = 1.0 / np.sqrt(D)

_CACHE = {}


def _build_nc(repeat: int = 1, mode: str = "full"):
    import concourse.bass as bass  # noqa: F401
    import concourse.tile as tile
    import concourse.mybir as mybir
    from concourse import bacc
    from concourse.masks import make_identity
    from contextlib import ExitStack

    f32 = mybir.dt.float32
    f32r = mybir.dt.float32r
    bf16 = mybir.dt.bfloat16

    nc = bacc.Bacc("TRN2", target_bir_lowering=False, debug=False,
                   num_devices=NCORES)

    x_d = nc.dram_tensor("x", [S, D], f32, kind="ExternalInput").ap()
    wk_d = nc.dram_tensor("Wk", [D, D], f32, kind="ExternalInput").ap()
    wo_d = nc.dram_tensor("Wo", [D, D], f32, kind="ExternalInput").ap()
    out_d = nc.dram_tensor("out", [SH, D], f32, kind="ExternalOutput").ap()

    NSC = S // P           # 16 sequence chunks
    NDC = D // P           # 4 feature chunks
    NQB = SH // 512        # 2 query blocks of 512
    QB = 512

    import contextlib
    with tile.TileContext(nc) as tc:
        loop_cm = tc.For_i(0, repeat, 1) if repeat > 1 else contextlib.nullcontext()
        with loop_cm, ExitStack() as ctx:
            consts = ctx.enter_context(tc.tile_pool(name="consts", bufs=1))
            kpool = ctx.enter_context(tc.tile_pool(name="kpool", bufs=1))
            epool = ctx.enter_context(
                tc.tile_pool(name="epool", bufs=(5 if mode == "ct" else 3)))
            vpool = ctx.enter_context(tc.tile_pool(name="vpool", bufs=1))
            opool = ctx.enter_context(tc.tile_pool(name="opool", bufs=2))
            # psum: tag A = 4 banks x1, tag B = 2 banks x2  -> 8 banks total
            ps = ctx.enter_context(tc.tile_pool(name="ps", bufs=1, space="PSUM"))

            def spanA():
                return ps.tile([P, 4, 512], f32, tag="A", name="spA")

            def spanB(shape=None, name="spB"):
                return ps.tile(shape or [P, 2, 512], f32, tag="B", bufs=2,
                               name=name)

            ident = consts.tile([P, P], f32)
            make_identity(nc, ident[:])

            ones1x64f = consts.tile([1, 64], f32)
            nc.gpsimd.memset(ones1x64f[:], 1.0)
            ones1x64 = consts.tile([1, 64], f32r)
            nc.vector.tensor_copy(ones1x64[:], ones1x64f[:])
            ones_bf = consts.tile([P, 1], bf16)
            nc.gpsimd.memset(ones_bf[:], 1.0)

            woT = consts.tile([P, NDC, 512], f32r)      # [d', e]
            k_bf = kpool.tile([P, NSC, 512], bf16)      # K [s, e]
            kones = (None if mode == "ct" else
                     kpool.tile([P, NSC, H, HD + 1], bf16))
            khT = kpool.tile([P, NDC, S], bf16)         # K^T [e, s]
            wvt = vpool.tile([P, NDC, SH], f32r)        # wVT (norm in place)

            # ---- phase 0/1: weights, x -> xT (PE transposes) -> K proj -----
            with tc.tile_pool(name="stage", bufs=1) as stage:
                wk_sb = stage.tile([P, NDC, 512], f32, tag="w")
                nc.sync.dma_start(wk_sb[:], wk_d.rearrange("(eo p) d -> p eo d", p=P))
                wkT_r = consts.tile([P, NDC, 512], f32r)
                sp = spanA()
                for dc in range(NDC):
                    for eo in range(NDC):
                        nc.tensor.transpose(
                            sp[:, dc, eo * P:(eo + 1) * P],
                            wk_sb[:, eo, dc * P:(dc + 1) * P], ident[:])
                nc.scalar.copy(wkT_r[:], sp[:])

                # x: 8 groups of 2 seq-chunks; PE transpose -> fp32r Kproj
                for g in range(8):
                    g0 = g * 2
                    x_g = stage.tile([P, 2, 512], f32, tag="x", bufs=3,
                                     name="x_g")
                    for i in range(2):
                        sc = g0 + i
                        nc.sync.dma_start(x_g[:, i, :],
                                          x_d[sc * P:(sc + 1) * P, :])
                    spt = spanB([P, 4, 256], name="sptr")
                    for i in range(2):
                        for dc in range(NDC):
                            nc.tensor.transpose(
                                spt[:, dc, i * P:(i + 1) * P],
                                x_g[:, i, dc * P:(dc + 1) * P], ident[:])
                    xT_g = stage.tile([P, NDC, 256], f32r, tag="xT", bufs=2,
                                      name="xT_g")
                    nc.scalar.copy(xT_g[:], spt[:])

                    spk = spanB(name="spkp")
                    for i in range(2):
                        for dc in range(NDC):
                            nc.tensor.matmul(
                                spk[:, i, :],
                                xT_g[:, dc, i * P:(i + 1) * P],
                                wkT_r[:, dc, :],
                                start=(dc == 0), stop=(dc == NDC - 1))
                    nc.vector.tensor_copy(k_bf[:, g0:g0 + 2, :], spk[:, 0:2, :])
                    if kones is not None:
                        nc.vector.tensor_copy(
                            kones[:, g0:g0 + 2, :, 0:HD],
                            spk[:, 0:2, :].rearrange("p g (h e) -> p g h e",
                                                     h=H))
                    if g in (3, 7):
                        # khT XBAR transposes in two clusters so head 0's
                        # scores can start while later groups still project
                        for sc in range(g0 - 6, g0 + 2):
                            nc.sync.dma_start_transpose(
                                khT[:, :, sc * P:(sc + 1) * P], k_bf[:, sc, :])

                # Wo: transpose via PE (fp32 -> fp32r), span A
                wo_sb = stage.tile([P, NDC, 512], f32, tag="w")
                nc.sync.dma_start(wo_sb[:], wo_d.rearrange("(eo p) d -> p eo d", p=P))
                sp = spanA()
                for dc in range(NDC):
                    for eo in range(NDC):
                        nc.tensor.transpose(
                            sp[:, dc, eo * P:(eo + 1) * P],
                            wo_sb[:, eo, dc * P:(dc + 1) * P], ident[:])
                nc.scalar.copy(woT[:], sp[:])

            if kones is not None:
                nc.gpsimd.memset(kones[:, :, :, HD:HD + 1], 1.0)

            if mode == "phase0":
                # consume everything so DCE cannot strip phase 0/1 work
                with tc.tile_pool(name="sink", bufs=1, space="DRAM") as sink:
                    snk1 = sink.tile([P, NSC, 512], bf16, name="snk1")
                    nc.sync.dma_start(snk1[:], k_bf[:])
                    snk2 = sink.tile([P, NDC, S], bf16, name="snk2")
                    nc.sync.dma_start(snk2[:], khT[:])
                    snk3 = sink.tile([P, NSC, H, HD + 1], bf16, name="snk3")
                    nc.sync.dma_start(snk3[:], kones[:])
                    snk4 = sink.tile([P, NDC, 512], f32, name="snk4")
                    nc.sync.dma_start(snk4[:], woT[:].bitcast(f32))
                o_sb0 = opool.tile([P, 2, 512], f32, tag="osb", name="o_sb0")
                nc.vector.tensor_copy(o_sb0[:, 0, :], khT[:, 0, 0:512])
                nc.vector.tensor_copy(o_sb0[:, 1, :], k_bf[:, 0, :])
                nc.sync.dma_start(
                    out_d[0:2 * P, :].rearrange("(two p) d -> p two d", p=P),
                    o_sb0[:])

            # ---- head loop (qb outer), software-pipelined PV ----------------
            kc_groups = [(0, "A"), (4, "B"), (6, "B"), (8, "A"),
                         (12, "B"), (14, "B")]

            def emit_pv(h, qb, e_t):
                hp = (h % 2) * HD
                ec = h // 2
                pv = spanB([HD + 1, 512], name="pv")
                for kc in range(NSC):
                    nc.tensor.matmul(
                        pv[:], kones[:, kc, h, :], e_t[:, kc, :],
                        start=(kc == 0), stop=(kc == NSC - 1))
                nc.vector.tensor_copy(
                    wvt[hp:hp + HD, ec, qb * QB:(qb + 1) * QB], pv[0:HD, :])
                recip_t = vpool.tile([1, 512], f32r, tag="recip", bufs=4,
                                     name="recip_t")
                with nc.allow_low_precision(reason="fp32r recip is fine"):
                    nc.vector.reciprocal(recip_t[:], pv[HD:HD + 1, :])
                bc = spanB([HD, 512], name="bc")
                nc.tensor.matmul(
                    bc[:], ones1x64[:], recip_t[:], start=True, stop=True)
                nc.vector.tensor_tensor(
                    wvt[hp:hp + HD, ec, qb * QB:(qb + 1) * QB],
                    wvt[hp:hp + HD, ec, qb * QB:(qb + 1) * QB],
                    bc[:], mybir.AluOpType.mult)

            def emit_outproj(qc0):
                # two q-chunks of 128 per pass, psum in a B slot
                po = spanB(name="po")
                for j in range(2):
                    qc = qc0 + j
                    for dc in range(NDC):
                        nc.tensor.matmul(
                            po[:, j, :],
                            wvt[:, dc, qc * P:(qc + 1) * P],
                            woT[:, dc, :],
                            start=(dc == 0), stop=(dc == NDC - 1))
                o_sb = opool.tile([P, 2, 512], f32, tag="osb", name="o_sb")
                nc.vector.tensor_copy(o_sb[:], po[:])
                nc.sync.dma_start(
                    out_d[qc0 * P:(qc0 + 2) * P, :].rearrange(
                        "(two p) d -> p two d", p=P),
                    o_sb[:])

            def emit_pv_pair(j, qb, e_lo, e_hi):
                # heads (2j, 2j+1) concurrently via PE column tiling
                pv = spanB([P, 512], name="pvp")
                for kc in range(NSC):
                    nc.tensor.matmul(
                        pv[0:HD, :],
                        k_bf[:, kc, (2 * j) * HD:(2 * j + 1) * HD],
                        e_lo[:, kc, :],
                        start=(kc == 0), stop=(kc == NSC - 1),
                        tile_position=(0, 0))
                    nc.tensor.matmul(
                        pv[HD:2 * HD, :],
                        k_bf[:, kc, (2 * j + 1) * HD:(2 * j + 2) * HD],
                        e_hi[:, kc, :],
                        start=(kc == 0), stop=(kc == NSC - 1),
                        tile_position=(0, HD))
                nc.vector.tensor_copy(
                    wvt[:, j, qb * QB:(qb + 1) * QB], pv[:])

            def emit_rs_quad(g, qb, e_ts):
                # rowsums of heads 4g..4g+3 via 4-way column tiling (M=1)
                rs = spanB([97, 512], name="rs")
                for kc in range(NSC):
                    for hi in range(4):
                        nc.tensor.matmul(
                            rs[32 * hi:32 * hi + 1, :],
                            ones_bf[:, 0:1],
                            e_ts[hi][:, kc, :],
                            start=(kc == 0), stop=(kc == NSC - 1),
                            tile_position=(0, 32 * hi))
                for hi in range(4):
                    h = 4 * g + hi
                    hp = (h % 2) * HD
                    ec = h // 2
                    recip_t = vpool.tile([1, 512], f32r, tag="recip", bufs=4,
                                         name="recip_t")
                    with nc.allow_low_precision(reason="fp32r recip is fine"):
                        nc.vector.reciprocal(recip_t[:],
                                             rs[32 * hi:32 * hi + 1, :])
                    bc = spanB([HD, 512], name="bc")
                    nc.tensor.matmul(
                        bc[:], ones1x64[:], recip_t[:], start=True, stop=True)
                    nc.vector.tensor_tensor(
                        wvt[hp:hp + HD, ec, qb * QB:(qb + 1) * QB],
                        wvt[hp:hp + HD, ec, qb * QB:(qb + 1) * QB],
                        bc[:], mybir.AluOpType.mult)

            if mode == "ct":
                for qb in range(NQB):
                    quad = []
                    for h in range(H):
                        hp = (h % 2) * HD
                        ec = h // 2
                        e_t = epool.tile([P, NSC, 512], bf16, tag="E",
                                         name="e_t")
                        for g0, kind in kc_groups:
                            gn = 4 if kind == "A" else 2
                            sp = spanA() if kind == "A" else spanB()
                            for i in range(gn):
                                kc = g0 + i
                                nc.tensor.matmul(
                                    sp[:, i, :],
                                    khT[hp:hp + HD, ec, kc * P:(kc + 1) * P],
                                    khT[hp:hp + HD, ec, qb * QB:(qb + 1) * QB],
                                    start=True, stop=True)
                            nc.scalar.activation(
                                e_t[:, g0:g0 + gn, :], sp[:, 0:gn, :],
                                mybir.ActivationFunctionType.Exp, scale=SCALE)
                        quad.append(e_t)
                        if h % 2 == 1:
                            emit_pv_pair(h // 2, qb, quad[-2], quad[-1])
                        if h % 4 == 3:
                            emit_rs_quad(h // 4, qb, quad)
                            quad = []
                    for qc0 in range(qb * 4, qb * 4 + 4, 2):
                        emit_outproj(qc0)
            else:
                pending = None
                backlog = []
                for qb in range(NQB if mode != "phase0" else 0):
                    for h in range(H):
                        hp = (h % 2) * HD
                        ec = h // 2
                        e_t = epool.tile([P, NSC, 512], bf16, tag="E", name="e_t")
                        for g0, kind in kc_groups:
                            gn = 4 if kind == "A" else 2
                            sp = spanA() if kind == "A" else spanB()
                            for i in range(gn):
                                kc = g0 + i
                                nc.tensor.matmul(
                                    sp[:, i, :],
                                    khT[hp:hp + HD, ec, kc * P:(kc + 1) * P],
                                    khT[hp:hp + HD, ec, qb * QB:(qb + 1) * QB],
                                    start=True, stop=True)
                            nc.scalar.activation(
                                e_t[:, g0:g0 + gn, :], sp[:, 0:gn, :],
                                mybir.ActivationFunctionType.Exp, scale=SCALE)
                        if pending is not None and mode == "full":
                            emit_pv(*pending)
                            if pending[0] == H - 1:
                                backlog.extend(
                                    range(pending[1] * 4,
                                          pending[1] * 4 + 4, 2))
                            elif backlog and pending[0] % 2 == 1:
                                emit_outproj(backlog.pop(0))
                        pending = (h, qb, e_t)
                if mode == "full":
                    emit_pv(*pending)
                    backlog.extend(
                        range(pending[1] * 4, pending[1] * 4 + 4, 2))
                    for qc0 in backlog:
                        emit_outproj(qc0)

    nc.compile()
    return nc


def _get_nc(repeat: int = 1, mode: str = "full"):
    key = ("nc", repeat, mode)
    if key not in _CACHE:
        _CACHE[key] = _build_nc(repeat, mode)
    return _CACHE[key]


def kernel(x: np.ndarray, Wk: np.ndarray, Wo: np.ndarray, _trace=False):
    from concourse import bass_utils

    nc = _get_nc()
    x = np.asarray(x, dtype=np.float32)
    Wk = np.ascontiguousarray(np.asarray(Wk, dtype=np.float32))
    Wo = np.ascontiguousarray(np.asarray(Wo, dtype=np.float32))

    in_maps = []
    for c in range(NCORES):
        b, half = c // 2, c % 2
        xb = x[b]
        if half:
            xb = np.roll(xb, -SH, axis=0)
        in_maps.append({"x": np.ascontiguousarray(xb), "Wk": Wk, "Wo": Wo})

    res = bass_utils.run_bass_kernel_spmd(
        nc, in_maps, core_ids=list(range(NCORES)), trace=_trace)

    out = np.empty((B, S, D), dtype=np.float32)
    for c in range(NCORES):
        b, half = c // 2, c % 2
        out[b, half * SH:(half + 1) * SH] = res.results[c]["out"]
    if _trace:
        _CACHE["last_results"] = res
    return out
